# revision 14
# baseline (speedup 1.0000x reference)
"""Trainium2 Bass kernel for nn_MAD_GCN (retrieval_knn), v2.

Strategy: shard the B=512 edges across 8 NeuronCores (64 edges each, no
collectives). Per core, 4 head-tiles of 128 columns (64 edges x 2 sides,
single head per tile):

  phase 1 (PE):   s[c,n] = 2*q_c.pos_n - pn2_n via ONE 65-row bf16 matmul
                  per 1024-chunk (pn2 folded in as a 65th contraction row).
  phase 2:        ACT evicts each chunk to bf16 and DMAs it to a DRAM score
                  table srd [128*128 rows, 128]; DVE/Pool compute per-128
                  block maxes from the evicted bf16. Top-9 blocks per column
                  (max8 / match_replace / max8 + 2x max_index over the 128
                  pooled values) provably contain the global top-9 scores.
                  The 9 winner blocks are fetched back per-partition with
                  dma_gather (wrapped int16 index list built via a
                  tile(eye16) PE matmul), and an exact merge over the
                  gathered 1152 values yields ranks 2..9 = the 8 neighbors
                  (rank 1 is the query itself) + their global node indices.
  phase 3:        one dma_gather per tile fetches fused rows
                  [pos(n,h) | adj[n,dst_e] | adj[src_e,n]] (192 f32); DVE
                  recomputes diff/contrib/distance exactly as the reference,
                  per-partition adj selection via an eye(128) mask, softmin
                  over 2K+8 sentinel samples, sigmoid, per-edge mean.

Host-side work is limited to sharding/layout: transposes, per-core gather
tables, and per-edge constant tiles.
"""

import numpy as np

import concourse.bass as bass
import concourse.bacc as bacc
import concourse.mybir as mybir
import concourse.tile as tile
from concourse.bass_utils import run_bass_kernel_spmd

F32 = mybir.dt.float32
BF16 = mybir.dt.bfloat16
I32 = mybir.dt.int32
I16 = mybir.dt.int16
U16 = mybir.dt.uint16
ALU = mybir.AluOpType
ACTF = mybir.ActivationFunctionType
AX = mybir.AxisListType

N_NODES = 16384
H = 4
D = 64
B = 512
K = 8
NUM_SENT = 8
SENT_DIST = 1.0
N_CORES = 8
BLOC = B // N_CORES      # 64 edges per core

W = 128                  # score block width
NBLK = N_NODES // W      # 128 blocks per column
CH = 1024                # eviction chunk width
NCH = N_NODES // CH      # 16 chunks per tile
NW = 9                   # winner blocks per column (ranks 1..9)

# kept for test.py compat
USE_F32R = False
PN2_BF16 = False

# fraction of blockmax chunks on DVE (rest on Pool/gpsimd)
DVE_BM = [0, 3, 6, 9, 12, 15]


def emit(nc, tc, n_nodes=N_NODES, stage=99):
    # ---- DRAM I/O ----
    dr = {}
    for h in range(H):
        dr[f"rhs_{h}"] = nc.dram_tensor(f"rhs_{h}", [65, n_nodes], BF16,
                                        kind="ExternalInput")
        dr[f"lhsT_{h}"] = nc.dram_tensor(f"lhsT_{h}", [65, 128], BF16,
                                         kind="ExternalInput")
        dr[f"q_{h}"] = nc.dram_tensor(f"q_{h}", [128, D], F32,
                                      kind="ExternalInput")
        dr[f"g_{h}"] = nc.dram_tensor(f"g_{h}", [128, D], F32,
                                      kind="ExternalInput")
    for h in range(H):
        dr[f"pag_{h}"] = nc.dram_tensor(f"pag_{h}", [2 * n_nodes, 128], F32,
                                        kind="ExternalInput")
    dr["sb"] = nc.dram_tensor("sb", [128, 1], F32, kind="ExternalInput")
    dr["l2"] = nc.dram_tensor("l2", [128, 128], F32, kind="ExternalInput")
    dr["gm"] = nc.dram_tensor("gm", [128, 64], F32, kind="ExternalInput")
    dr["eye"] = nc.dram_tensor("eye", [128, 128], F32, kind="ExternalInput")
    dr["lw"] = nc.dram_tensor("lw", [128, 1], F32, kind="ExternalInput")
    pred = nc.dram_tensor("pred", [BLOC, 1], F32, kind="ExternalOutput")

    ctx = getattr(tc, "_emit_ctx")
    pc = ctx.enter_context(tc.tile_pool(name="const", bufs=1))
    pr = ctx.enter_context(tc.tile_pool(name="rhs", bufs=2))
    pw = ctx.enter_context(tc.tile_pool(name="work", bufs=2))
    pf = ctx.enter_context(tc.tile_pool(name="ph3", bufs=1))
    pps = ctx.enter_context(tc.tile_pool(name="psum", bufs=2, space="PSUM"))
    ppw = ctx.enter_context(tc.tile_pool(name="psumw", bufs=1, space="PSUM"))
    pdr = ctx.enter_context(tc.tile_pool(name="dram", bufs=2, space="DRAM"))

    # ---- consts ----
    lhsT = {}
    qt = {}
    gt = {}
    for h in range(H):
        lhsT[h] = pc.tile([65, 128], BF16, tag=f"lhsT{h}", name=f"lhsT{h}")
        nc.sync.dma_start(lhsT[h], dr[f"lhsT_{h}"][:, :])
        qt[h] = pc.tile([128, D], F32, tag=f"q{h}", name=f"q{h}")
        nc.sync.dma_start(qt[h], dr[f"q_{h}"][:, :])
        gt[h] = pc.tile([128, D], F32, tag=f"g{h}", name=f"g{h}")
        nc.sync.dma_start(gt[h], dr[f"g_{h}"][:, :])
    l2 = pc.tile([128, 128], F32, tag="l2", name="l2")
    nc.sync.dma_start(l2, dr["l2"][:, :])
    gm = pc.tile([128, 64], F32, tag="gm", name="gm")
    nc.sync.dma_start(gm, dr["gm"][:, :])
    eye = pc.tile([128, 64], F32, tag="eye", name="eye")
    nc.sync.dma_start(eye, dr["eye"][:, 0:64])
    lw = pc.tile([128, 1], F32, tag="lw", name="lw")
    nc.sync.dma_start(lw, dr["lw"][:, :])
    sb = pc.tile([128, 1], F32, tag="sb", name="sb")
    nc.sync.dma_start(sb, dr["sb"][:, :])

    piota = pc.tile([128, 1], I32, tag="piota", name="piota")
    nc.gpsimd.iota(piota, pattern=[[1, 1]], base=0, channel_multiplier=NBLK)
    piotaf = pc.tile([128, 1], F32, tag="piotaf", name="piotaf")
    nc.vector.tensor_copy(piotaf, piota)
    iota9_i = pc.tile([128, NW], I32, tag="iota9i", name="iota9i")
    nc.gpsimd.iota(iota9_i, pattern=[[1, NW]], base=0, channel_multiplier=0)
    iota9 = pc.tile([128, NW], F32, tag="iota9", name="iota9")
    nc.vector.tensor_copy(iota9, iota9_i)

    Racc = pf.tile([BLOC, H], F32, tag="Racc", name="Racc")

    def wrap_idx(tag, gif, ncols):
        """Build the wrapped+replicated int16 index list for dma_gather.

        gif: [128, ncols] f32 row indices (<= 32767). Returns an i16 tile
        [128, ncols*8] laid out so that list position i = c*128+p sits at
        (partition i%16, slot i//16), replicated into all 8 groups."""
        Rm = pw.tile([128, ncols * 8], F32, tag=f"R{tag}", name=f"R{tag}")
        nc.vector.tensor_mul(
            Rm[:, :].rearrange("p (c g) -> p c g", g=8),
            gif[:, :].rearrange("p (c one) -> p c one", one=1)
                     .to_broadcast([128, ncols, 8]),
            gm[:, 0:ncols * 8].rearrange("p (c g) -> p c g", g=8))
        psw = ppw.tile([128, 64], F32, tag=f"psw", name=f"psw{tag}", bufs=3)
        nc.tensor.matmul(psw[:, 0:ncols * 8], l2[:, :], Rm[:, :],
                         start=True, stop=True)
        idxf = pw.tile([128, ncols * 8], F32, tag=f"xf{tag}", name=f"xf{tag}")
        nc.scalar.copy(idxf, psw[:, 0:ncols * 8])
        idx16 = pw.tile([128, ncols * 8], I16, tag=f"x6{tag}", name=f"x6{tag}")
        nc.vector.tensor_copy(idx16, idxf)
        return idx16

    for h in range(H):
        rhs = pr.tile([65, n_nodes], BF16, tag="rhs", name="rhs")
        nc.sync.dma_start(rhs, dr[f"rhs_{h}"][:, :])
        srd = pdr.tile([128 * NBLK, W], BF16, tag="srd", name="srd")
        srd3 = srd[:, :].rearrange("(p b) w -> p b w", p=128)
        pooled = pw.tile([128, NBLK], BF16, tag="pooled", name="pooled")

        # ---- phase 1+2a: matmul, evict, store, blockmax ----
        for c in range(NCH):
            ps = pps.tile([128, CH], F32, tag="ps", name="ps")
            for j in range(CH // 512):
                nc.tensor.matmul(ps[:, j * 512:(j + 1) * 512],
                                 lhsT[h][:, :],
                                 rhs[:, c * CH + j * 512:c * CH + (j + 1) * 512],
                                 start=True, stop=True)
            sch = pw.tile([128, CH], BF16, tag="sch", name="sch", bufs=3)
            nc.scalar.copy(sch, ps[:, :])
            bpc = CH // W  # blocks per chunk
            nc.sync.dma_start(srd3[:, c * bpc:(c + 1) * bpc, :],
                              sch[:, :].rearrange("p (b w) -> p b w", w=W))
            nc.vector.tensor_reduce(pooled[:, c * bpc:(c + 1) * bpc],
                                    sch[:, :].rearrange(
                                        "p (b w) -> p b w", w=W),
                                    axis=AX.X, op=ALU.max)

        # ---- phase 2b: top-9 blocks ----
        pm1 = pw.tile([128, 8], BF16, tag="pm1", name="pm1")
        nc.vector.max(pm1, pooled)
        prep = pw.tile([128, NBLK], BF16, tag="prep", name="prep")
        nc.vector.match_replace(out=prep, in_to_replace=pm1, in_values=pooled,
                                imm_value=-3.0e38)
        pm2 = pw.tile([128, 8], BF16, tag="pm2", name="pm2")
        nc.vector.max(pm2, prep)
        bidxA = pw.tile([128, 8], U16, tag="bidxA", name="bidxA")
        nc.vector.max_index(bidxA, pm1, pooled)
        bidxB = pw.tile([128, 8], U16, tag="bidxB", name="bidxB")
        nc.vector.max_index(bidxB, pm2, prep)

        bAf = pw.tile([128, 8], F32, tag="bAf", name="bAf")
        nc.vector.tensor_copy(bAf, bidxA)
        bBf = pw.tile([128, 1], F32, tag="bBf", name="bBf")
        nc.vector.tensor_copy(bBf, bidxB[:, 0:1])
        giA = pw.tile([128, 8], F32, tag="giA", name="giA")
        nc.vector.tensor_scalar(giA, bAf, piotaf[:, :], None, op0=ALU.add)
        giB = pw.tile([128, 1], F32, tag="giB", name="giB")
        nc.vector.tensor_scalar(giB, bBf, piotaf[:, :], None, op0=ALU.add)

        idxA = wrap_idx(f"A", giA, 8)
        idxB = wrap_idx(f"B", giB, 1)

        gat = pw.tile([128, NW, W], BF16, tag="gat", name="gat")
        nc.gpsimd.dma_gather(gat[:, 0:8, :], srd[:, :], idxA[:, :],
                             1024, 1024, W)
        nc.gpsimd.dma_gather(gat[:, 8:9, :], srd[:, :], idxB[:, :],
                             128, 128, W)
        gatf = gat[:, :, :].rearrange("p a b -> p (a b)")

        # ---- phase 2c: exact ranks 1..9 + global node indices ----
        em1 = pw.tile([128, 8], BF16, tag="em1", name="em1")
        nc.vector.max(em1, gatf)
        erep = pw.tile([128, NW * W], BF16, tag="erep", name="erep")
        nc.vector.match_replace(out=erep, in_to_replace=em1, in_values=gatf,
                                imm_value=-3.0e38)
        em2 = pw.tile([128, 8], BF16, tag="em2", name="em2")
        nc.vector.max(em2, erep)
        ei1 = pw.tile([128, 8], U16, tag="ei1", name="ei1")
        nc.vector.max_index(ei1, em1, gatf)
        ei2 = pw.tile([128, 8], U16, tag="ei2", name="ei2")
        nc.vector.max_index(ei2, em2, erep)

        eidx = pw.tile([128, 8], I32, tag="eidx", name="eidx")
        nc.vector.tensor_copy(eidx[:, 0:7], ei1[:, 1:8])
        nc.vector.tensor_copy(eidx[:, 7:8], ei2[:, 0:1])
        blk = pw.tile([128, 8], I32, tag="blk", name="blk")
        nc.vector.tensor_scalar(blk, eidx, 7, None,
                                op0=ALU.logical_shift_right)
        off = pw.tile([128, 8], I32, tag="off", name="off")
        nc.vector.tensor_scalar(off, eidx, W - 1, None, op0=ALU.bitwise_and)
        blkf = pw.tile([128, 8], F32, tag="blkf", name="blkf")
        nc.vector.tensor_copy(blkf, blk)
        offf = pw.tile([128, 8], F32, tag="offf", name="offf")
        nc.vector.tensor_copy(offf, off)
        # bidx9 values per winner slot via eq-match over the 9 blocks
        bid9 = pw.tile([128, NW], F32, tag="bid9", name="bid9")
        nc.vector.tensor_copy(bid9[:, 0:8], bAf)
        nc.vector.tensor_copy(bid9[:, 8:9], bBf)
        eqm = pw.tile([128, 8, NW], F32, tag="eqm", name="eqm")
        nc.vector.tensor_tensor(
            eqm,
            iota9[:, :].rearrange("p (one j) -> p one j", one=1)
                       .to_broadcast([128, 8, NW]),
            blkf[:, :].rearrange("p (k one) -> p k one", one=1)
                      .to_broadcast([128, 8, NW]),
            op=ALU.is_equal)
        nc.vector.tensor_mul(
            eqm, eqm,
            bid9[:, :].rearrange("p (one j) -> p one j", one=1)
                      .to_broadcast([128, 8, NW]))
        bsf = pw.tile([128, 8], F32, tag="bsf", name="bsf")
        nc.vector.reduce_sum(bsf, eqm[:, :, :], axis=AX.X)
        # global node index = block*W + off  (as f32, exact)
        n32f = pw.tile([128, 8], F32, tag="n32f", name="n32f")
        nc.vector.tensor_scalar(n32f, bsf, float(W), None, op0=ALU.mult)
        nc.vector.tensor_add(n32f, n32f, offf)
        if stage <= 2:
            nc.sync.dma_start(pred[:, :], n32f[0:BLOC, 0:1])
            return pred

        # ---- phase 3 ----
        pagf = pw.tile([128, 8], F32, tag="pagf", name="pagf")
        nc.vector.tensor_scalar(pagf, n32f, sb[:, :], None, op0=ALU.add)
        idxC = wrap_idx(f"C", pagf, 8)
        pg = pw.tile([128, 8, 128], F32, tag="pg", name="pg")
        nc.gpsimd.dma_gather(pg[:, :, :], dr[f"pag_{h}"][:, :],
                             idxC[:, :], 1024, 1024, 128)

        qb = qt[h][:, :].rearrange("p (one d) -> p one d", one=1) \
                        .to_broadcast([128, 8, D])
        gb = gt[h][:, :].rearrange("p (one d) -> p one d", one=1) \
                        .to_broadcast([128, 8, D])
        diff = pw.tile([128, 8, D], F32, tag="diff", name="diff")
        nc.vector.tensor_sub(diff, qb, pg[:, :, 0:D])
        prod = pw.tile([128, 8, D], F32, tag="prod", name="prod")
        nc.vector.tensor_mul(prod, diff, gb)
        contrib = pw.tile([128, 8], F32, tag="contrib", name="contrib")
        nc.vector.reduce_sum(contrib, prod[:, :, :], axis=AX.X)
        nc.vector.tensor_mul(prod, diff, diff)
        d2 = pw.tile([128, 8], F32, tag="d2", name="d2")
        nc.vector.reduce_sum(d2, prod[:, :, :], axis=AX.X)
        dist = pw.tile([128, 8], F32, tag="dist", name="dist")
        nc.scalar.sqrt(dist, d2)
        adjm = pw.tile([128, 8, 64], F32, tag="adjm", name="adjm")
        nc.vector.tensor_mul(
            adjm, pg[:, :, D:128],
            eye[:, :].rearrange("p (one j) -> p one j", one=1)
                     .to_broadcast([128, 8, 64]))
        adjv = pw.tile([128, 8], F32, tag="adjv", name="adjv")
        nc.vector.reduce_sum(adjv, adjm[:, :, :], axis=AX.X)
        nc.vector.tensor_scalar(adjv, adjv, lw[:, :], None, op0=ALU.mult)
        nc.vector.tensor_add(adjv, adjv, contrib)
        # adjv now holds logits [128, 8]
        if stage <= 3:
            nc.sync.dma_start(pred[:, :], adjv[0:BLOC, 0:1])
            return pred

        dfull = pw.tile([BLOC, 2 * K + NUM_SENT], F32, tag="dfull",
                        name="dfull")
        lfull = pw.tile([BLOC, 2 * K + NUM_SENT], F32, tag="lfull",
                        name="lfull")
        nc.vector.tensor_copy(dfull[:, 0:8], dist[0:BLOC, :])
        nc.sync.dma_start(dfull[:, 8:16], dist[BLOC:128, :])
        nc.vector.memset(dfull[:, 16:24], SENT_DIST)
        nc.vector.tensor_copy(lfull[:, 0:8], adjv[0:BLOC, :])
        nc.sync.dma_start(lfull[:, 8:16], adjv[BLOC:128, :])
        nc.vector.memset(lfull[:, 16:24], 0.0)

        mn = pw.tile([BLOC, 1], F32, tag="mn", name="mn")
        nc.vector.tensor_reduce(mn, dfull[:, :], axis=AX.X, op=ALU.min)
        e24 = pw.tile([BLOC, 24], F32, tag="e24", name="e24")
        nc.scalar.activation(e24, dfull[:, :], ACTF.Exp, bias=mn[:, :],
                             scale=-1.0)
        z = pw.tile([BLOC, 1], F32, tag="z", name="z")
        nc.vector.reduce_sum(z, e24[:, :], axis=AX.X)
        el = pw.tile([BLOC, 24], F32, tag="el", name="el")
        nc.vector.tensor_mul(el, e24, lfull)
        wl = pw.tile([BLOC, 1], F32, tag="wl", name="wl")
        nc.vector.reduce_sum(wl, el[:, :], axis=AX.X)
        rz = pw.tile([BLOC, 1], F32, tag="rz", name="rz")
        nc.vector.reciprocal(rz, z)
        nc.vector.tensor_mul(Racc[:, h:h + 1], wl, rz)

    rs = pf.tile([BLOC, 1], F32, tag="rs", name="rs")
    nc.vector.reduce_sum(rs, Racc[:, :], axis=AX.X)
    outv = pf.tile([BLOC, 1], F32, tag="outv", name="outv")
    nc.scalar.activation(outv, rs, ACTF.Sigmoid, scale=1.0 / H)
    nc.sync.dma_start(pred[:, :], outv)
    return pred


def build_nc(n_nodes=N_NODES, use_f32r=USE_F32R, pn2_bf16=PN2_BF16, stage=99):
    from contextlib import ExitStack
    nc = bacc.Bacc("TRN2", target_bir_lowering=False, debug=False)
    with tile.TileContext(nc) as tc:
        with ExitStack() as ctx:
            tc._emit_ctx = ctx
            emit(nc, tc, n_nodes, stage=stage)
    nc.compile()
    return nc


def host_prep(core, pos, grads, adj, label_w, edges, n_nodes=N_NODES,
              pn2_bf16=PN2_BF16):
    import ml_dtypes
    bf = ml_dtypes.bfloat16
    b0 = core * BLOC
    src = np.asarray(edges[0, b0:b0 + BLOC], np.int64)
    dst = np.asarray(edges[1, b0:b0 + BLOC], np.int64)
    pn2 = np.sum(pos * pos, axis=2)  # (N, H) f32

    im = {}
    for h in range(H):
        r = np.empty((65, n_nodes), np.float32)
        r[0:64] = pos[:, h, :].T
        r[64] = -pn2[:, h]
        im[f"rhs_{h}"] = r.astype(bf)
        L = np.empty((65, 128), np.float32)
        L[0:64, 0:64] = 2.0 * pos[src, h, :].T
        L[0:64, 64:128] = 2.0 * pos[dst, h, :].T
        L[64, :] = 1.0
        im[f"lhsT_{h}"] = L.astype(bf)
        q = np.empty((128, D), np.float32)
        q[0:64] = pos[src, h, :]
        q[64:128] = pos[dst, h, :]
        im[f"q_{h}"] = q
        g = np.empty((128, D), np.float32)
        g[0:64] = grads[dst, h, :]
        g[64:128] = grads[src, h, :]
        im[f"g_{h}"] = g
    adjcol = np.ascontiguousarray(adj[:, dst])          # (N, 64)
    adjrT = np.ascontiguousarray(adj[src, :].T)         # (N, 64)
    for h in range(H):
        pag = np.empty((2 * n_nodes, 128), np.float32)
        pag[0:n_nodes, 0:64] = pos[:, h, :]
        pag[0:n_nodes, 64:128] = adjcol
        pag[n_nodes:, 0:64] = pos[:, h, :]
        pag[n_nodes:, 64:128] = adjrT
        im[f"pag_{h}"] = pag
    im["l2"] = np.tile(np.eye(16, dtype=np.float32), (8, 8))
    gmv = np.zeros((128, 64), np.float32)
    kk = np.arange(128)
    for c in range(8):
        gmv[kk, c * 8 + kk // 16] = 1.0
    im["gm"] = gmv
    ey = np.zeros((128, 128), np.float32)
    ey[kk, kk % 64] = 1.0
    im["eye"] = ey
    sbv = np.zeros((128, 1), np.float32)
    sbv[64:, 0] = float(n_nodes)
    im["sb"] = sbv
    im["lw"] = np.full((128, 1), float(np.asarray(label_w).reshape(-1)[0]),
                       np.float32)
    return im


_NC_CACHE = {}


def kernel(pos, grads, adj, label_w, edges):
    pos = np.asarray(pos, np.float32)
    grads = np.asarray(grads, np.float32)
    adj = np.asarray(adj, np.float32)
    label_w = np.asarray(label_w, np.float32)
    edges_np = np.asarray(edges)

    key = (N_NODES,)
    if key not in _NC_CACHE:
        _NC_CACHE[key] = build_nc(N_NODES)
    nc = _NC_CACHE[key]

    in_maps = [host_prep(r, pos, grads, adj, label_w, edges_np, N_NODES)
               for r in range(N_CORES)]
    res = run_bass_kernel_spmd(nc, in_maps, core_ids=list(range(N_CORES)))
    out = np.concatenate([res.results[r]["pred"][:, 0]
                          for r in range(N_CORES)])
    return out.astype(np.float32)


# revision 18
# speedup vs baseline: 1.5151x; 1.5151x over previous
"""Trainium2 Bass kernel for nn_MAD_GCN (retrieval_knn), v2.

Strategy: shard the B=512 edges across 8 NeuronCores (64 edges each, no
collectives). Per core, 4 head-tiles of 128 columns (64 edges x 2 sides,
single head per tile):

  phase 1 (PE):   s[c,n] = 2*q_c.pos_n - pn2_n via ONE 65-row bf16 matmul
                  per 1024-chunk (pn2 folded in as a 65th contraction row).
  phase 2:        ACT evicts each chunk to bf16 and DMAs it to a DRAM score
                  table srd [128*128 rows, 128]; DVE/Pool compute per-128
                  block maxes from the evicted bf16. Top-9 blocks per column
                  (max8 / match_replace / max8 + 2x max_index over the 128
                  pooled values) provably contain the global top-9 scores.
                  The 9 winner blocks are fetched back per-partition with
                  dma_gather (wrapped int16 index list built via a
                  tile(eye16) PE matmul), and an exact merge over the
                  gathered 1152 values yields ranks 2..9 = the 8 neighbors
                  (rank 1 is the query itself) + their global node indices.
  phase 3:        one dma_gather per tile fetches fused rows
                  [pos(n,h) | adj[n,dst_e] | adj[src_e,n]] (192 f32); DVE
                  recomputes diff/contrib/distance exactly as the reference,
                  per-partition adj selection via an eye(128) mask, softmin
                  over 2K+8 sentinel samples, sigmoid, per-edge mean.

Host-side work is limited to sharding/layout: transposes, per-core gather
tables, and per-edge constant tiles.
"""

import numpy as np

import concourse.bass as bass
import concourse.bacc as bacc
import concourse.mybir as mybir
import concourse.tile as tile
from concourse.bass_utils import run_bass_kernel_spmd

F32 = mybir.dt.float32
BF16 = mybir.dt.bfloat16
I32 = mybir.dt.int32
I16 = mybir.dt.int16
U16 = mybir.dt.uint16
ALU = mybir.AluOpType
ACTF = mybir.ActivationFunctionType
AX = mybir.AxisListType

N_NODES = 16384
H = 4
D = 64
B = 512
K = 8
NUM_SENT = 8
SENT_DIST = 1.0
N_CORES = 8
BLOC = B // N_CORES      # 64 edges per core

W = 128                  # score block width
NBLK = N_NODES // W      # 128 blocks per column
CH = 1024                # eviction chunk width
NCH = N_NODES // CH      # 16 chunks per tile
NW = 9                   # winner blocks per column (ranks 1..9)

# kept for test.py compat
USE_F32R = False
PN2_BF16 = False

# fraction of blockmax chunks on DVE (rest on Pool/gpsimd)
DVE_BM = [0, 3, 6, 9, 12, 15]


def emit(nc, tc, n_nodes=N_NODES, stage=99, pfx="", dr=None):
    # ---- DRAM I/O ----
    if dr is None:
        dr = {}
        for h in range(H):
            dr[f"rhs_{h}"] = nc.dram_tensor(f"{pfx}rhs_{h}", [65, n_nodes],
                                            BF16, kind="ExternalInput")
            dr[f"lhsT_{h}"] = nc.dram_tensor(f"{pfx}lhsT_{h}", [65, 128],
                                             BF16, kind="ExternalInput")
            dr[f"q_{h}"] = nc.dram_tensor(f"{pfx}q_{h}", [128, D], F32,
                                          kind="ExternalInput")
            dr[f"g_{h}"] = nc.dram_tensor(f"{pfx}g_{h}", [128, D], F32,
                                          kind="ExternalInput")
        for h in range(H):
            dr[f"pag_{h}"] = nc.dram_tensor(f"{pfx}pag_{h}",
                                            [2 * n_nodes, 128], F32,
                                            kind="ExternalInput")
        dr["sb"] = nc.dram_tensor(pfx + "sb", [128, 1], F32,
                                  kind="ExternalInput")
        dr["l2"] = nc.dram_tensor(pfx + "l2", [128, 128], F32,
                                  kind="ExternalInput")
        dr["gm"] = nc.dram_tensor(pfx + "gm", [128, 64], F32,
                                  kind="ExternalInput")
        dr["eye"] = nc.dram_tensor(pfx + "eye", [128, 128], F32,
                                   kind="ExternalInput")
        dr["lw"] = nc.dram_tensor(pfx + "lw", [128, 1], F32,
                                  kind="ExternalInput")
        dr["pred"] = nc.dram_tensor(pfx + "pred", [BLOC, 1], F32,
                                    kind="ExternalOutput")
    pred = dr["pred"]

    ctx = getattr(tc, "_emit_ctx")
    pc = ctx.enter_context(tc.tile_pool(name=pfx + "const", bufs=1))
    pr = ctx.enter_context(tc.tile_pool(name=pfx + "rhs", bufs=2))
    pw = ctx.enter_context(tc.tile_pool(name=pfx + "work", bufs=2))
    pf = ctx.enter_context(tc.tile_pool(name=pfx + "ph3", bufs=1))
    pps = ctx.enter_context(tc.tile_pool(name=pfx + "psum", bufs=2, space="PSUM"))
    ppw = ctx.enter_context(tc.tile_pool(name=pfx + "psumw", bufs=1, space="PSUM"))
    pdr = ctx.enter_context(tc.tile_pool(name=pfx + "dram", bufs=2, space="DRAM"))

    # ---- consts ----
    lhsT = {}
    qt = {}
    gt = {}
    for h in range(H):
        lhsT[h] = pc.tile([65, 128], BF16, tag=f"lhsT{h}", name=f"lhsT{h}")
        nc.sync.dma_start(lhsT[h], dr[f"lhsT_{h}"][:, :])
        qt[h] = pc.tile([128, D], F32, tag=f"q{h}", name=f"q{h}")
        nc.sync.dma_start(qt[h], dr[f"q_{h}"][:, :])
        gt[h] = pc.tile([128, D], F32, tag=f"g{h}", name=f"g{h}")
        nc.sync.dma_start(gt[h], dr[f"g_{h}"][:, :])
    l2 = pc.tile([128, 128], F32, tag="l2", name="l2")
    nc.sync.dma_start(l2, dr["l2"][:, :])
    gm = pc.tile([128, 64], F32, tag="gm", name="gm")
    nc.sync.dma_start(gm, dr["gm"][:, :])
    eye = pc.tile([128, 64], F32, tag="eye", name="eye")
    nc.sync.dma_start(eye, dr["eye"][:, 0:64])
    lw = pc.tile([128, 1], F32, tag="lw", name="lw")
    nc.sync.dma_start(lw, dr["lw"][:, :])
    sb = pc.tile([128, 1], F32, tag="sb", name="sb")
    nc.sync.dma_start(sb, dr["sb"][:, :])

    piota = pc.tile([128, 1], I32, tag="piota", name="piota")
    nc.gpsimd.iota(piota, pattern=[[1, 1]], base=0, channel_multiplier=NBLK)
    piotaf = pc.tile([128, 1], F32, tag="piotaf", name="piotaf")
    nc.vector.tensor_copy(piotaf, piota)
    iota9_i = pc.tile([128, NW], I32, tag="iota9i", name="iota9i")
    nc.gpsimd.iota(iota9_i, pattern=[[1, NW]], base=0, channel_multiplier=0)
    iota9 = pc.tile([128, NW], F32, tag="iota9", name="iota9")
    nc.vector.tensor_copy(iota9, iota9_i)

    Racc = pf.tile([BLOC, H], F32, tag="Racc", name="Racc")

    def wrap_idx(tag, gif, ncols):
        """Build the wrapped+replicated int16 index list for dma_gather.

        gif: [128, ncols] f32 row indices (<= 32767). Returns an i16 tile
        [128, ncols*8] laid out so that list position i = c*128+p sits at
        (partition i%16, slot i//16), replicated into all 8 groups."""
        Rm = pw.tile([128, ncols * 8], F32, tag=f"R{tag}", name=f"R{tag}")
        nc.vector.tensor_mul(
            Rm[:, :].rearrange("p (c g) -> p c g", g=8),
            gif[:, :].rearrange("p (c one) -> p c one", one=1)
                     .to_broadcast([128, ncols, 8]),
            gm[:, 0:ncols * 8].rearrange("p (c g) -> p c g", g=8))
        psw = ppw.tile([128, 64], F32, tag=f"psw", name=f"psw{tag}", bufs=3)
        nc.tensor.matmul(psw[:, 0:ncols * 8], l2[:, :], Rm[:, :],
                         start=True, stop=True)
        idxf = pw.tile([128, ncols * 8], F32, tag=f"xf{tag}", name=f"xf{tag}")
        nc.scalar.copy(idxf, psw[:, 0:ncols * 8])
        idx16 = pw.tile([128, ncols * 8], I16, tag=f"x6{tag}", name=f"x6{tag}")
        nc.vector.tensor_copy(idx16, idxf)
        return idx16

    for h in range(H):
        rhs = pr.tile([65, n_nodes], BF16, tag="rhs", name="rhs")
        nc.sync.dma_start(rhs, dr[f"rhs_{h}"][:, :])
        srd = pdr.tile([128 * NBLK, W], BF16, tag="srd", name="srd")
        srd3 = srd[:, :].rearrange("(p b) w -> p b w", p=128)
        pooled = pw.tile([128, NBLK], BF16, tag="pooled", name="pooled")

        # ---- phase 1+2a: matmul, evict, store, blockmax ----
        for c in range(NCH):
            ps = pps.tile([128, CH], F32, tag="ps", name="ps")
            for j in range(CH // 512):
                nc.tensor.matmul(ps[:, j * 512:(j + 1) * 512],
                                 lhsT[h][:, :],
                                 rhs[:, c * CH + j * 512:c * CH + (j + 1) * 512],
                                 start=True, stop=True)
            sch = pw.tile([128, CH], BF16, tag="sch", name="sch", bufs=3)
            nc.scalar.copy(sch, ps[:, :])
            bpc = CH // W  # blocks per chunk
            nc.sync.dma_start(srd3[:, c * bpc:(c + 1) * bpc, :],
                              sch[:, :].rearrange("p (b w) -> p b w", w=W))
            nc.vector.tensor_reduce(pooled[:, c * bpc:(c + 1) * bpc],
                                    sch[:, :].rearrange(
                                        "p (b w) -> p b w", w=W),
                                    axis=AX.X, op=ALU.max)

        # ---- phase 2b: top-9 blocks ----
        if stage < 2:
            continue
        pm1 = pw.tile([128, 8], BF16, tag="pm1", name="pm1")
        nc.vector.max(pm1, pooled)
        prep = pw.tile([128, NBLK], BF16, tag="prep", name="prep")
        nc.vector.match_replace(out=prep, in_to_replace=pm1, in_values=pooled,
                                imm_value=-3.0e38)
        pm2 = pw.tile([128, 8], BF16, tag="pm2", name="pm2")
        nc.vector.max(pm2, prep)
        bidxA = pw.tile([128, 8], U16, tag="bidxA", name="bidxA")
        nc.vector.max_index(bidxA, pm1, pooled)
        bidxB = pw.tile([128, 8], U16, tag="bidxB", name="bidxB")
        nc.vector.max_index(bidxB, pm2, prep)

        bAf = pw.tile([128, 8], F32, tag="bAf", name="bAf")
        nc.vector.tensor_copy(bAf, bidxA)
        bBf = pw.tile([128, 1], F32, tag="bBf", name="bBf")
        nc.vector.tensor_copy(bBf, bidxB[:, 0:1])
        giA = pw.tile([128, 8], F32, tag="giA", name="giA")
        nc.vector.tensor_scalar(giA, bAf, piotaf[:, :], None, op0=ALU.add)
        giB = pw.tile([128, 1], F32, tag="giB", name="giB")
        nc.vector.tensor_scalar(giB, bBf, piotaf[:, :], None, op0=ALU.add)

        idxA = wrap_idx(f"A", giA, 8)
        idxB = wrap_idx(f"B", giB, 1)

        if stage < 3:
            continue
        gat = pw.tile([128, NW, W], BF16, tag="gat", name="gat")
        nc.gpsimd.dma_gather(gat[:, 0:8, :], srd[:, :], idxA[:, :],
                             1024, 1024, W)
        nc.gpsimd.dma_gather(gat[:, 8:9, :], srd[:, :], idxB[:, :],
                             128, 128, W)
        gatf = gat[:, :, :].rearrange("p a b -> p (a b)")

        # ---- phase 2c: exact ranks 1..9 + global node indices ----
        em1 = pw.tile([128, 8], BF16, tag="em1", name="em1")
        nc.vector.max(em1, gatf)
        erep = pw.tile([128, NW * W], BF16, tag="erep", name="erep")
        nc.vector.match_replace(out=erep, in_to_replace=em1, in_values=gatf,
                                imm_value=-3.0e38)
        em2 = pw.tile([128, 8], BF16, tag="em2", name="em2")
        nc.vector.max(em2, erep)
        ei1 = pw.tile([128, 8], U16, tag="ei1", name="ei1")
        nc.vector.max_index(ei1, em1, gatf)
        ei2 = pw.tile([128, 8], U16, tag="ei2", name="ei2")
        nc.vector.max_index(ei2, em2, erep)

        eidx = pw.tile([128, 8], I32, tag="eidx", name="eidx")
        nc.vector.tensor_copy(eidx[:, 0:7], ei1[:, 1:8])
        nc.vector.tensor_copy(eidx[:, 7:8], ei2[:, 0:1])
        blk = pw.tile([128, 8], I32, tag="blk", name="blk")
        nc.vector.tensor_scalar(blk, eidx, 7, None,
                                op0=ALU.logical_shift_right)
        off = pw.tile([128, 8], I32, tag="off", name="off")
        nc.vector.tensor_scalar(off, eidx, W - 1, None, op0=ALU.bitwise_and)
        blkf = pw.tile([128, 8], F32, tag="blkf", name="blkf")
        nc.vector.tensor_copy(blkf, blk)
        offf = pw.tile([128, 8], F32, tag="offf", name="offf")
        nc.vector.tensor_copy(offf, off)
        # bidx9 values per winner slot via eq-match over the 9 blocks
        bid9 = pw.tile([128, NW], F32, tag="bid9", name="bid9")
        nc.vector.tensor_copy(bid9[:, 0:8], bAf)
        nc.vector.tensor_copy(bid9[:, 8:9], bBf)
        eqm = pw.tile([128, 8, NW], F32, tag="eqm", name="eqm")
        nc.vector.tensor_tensor(
            eqm,
            iota9[:, :].rearrange("p (one j) -> p one j", one=1)
                       .to_broadcast([128, 8, NW]),
            blkf[:, :].rearrange("p (k one) -> p k one", one=1)
                      .to_broadcast([128, 8, NW]),
            op=ALU.is_equal)
        nc.vector.tensor_mul(
            eqm, eqm,
            bid9[:, :].rearrange("p (one j) -> p one j", one=1)
                      .to_broadcast([128, 8, NW]))
        bsf = pw.tile([128, 8], F32, tag="bsf", name="bsf")
        nc.vector.reduce_sum(bsf, eqm[:, :, :], axis=AX.X)
        # global node index = block*W + off  (as f32, exact)
        n32f = pw.tile([128, 8], F32, tag="n32f", name="n32f")
        nc.vector.tensor_scalar(n32f, bsf, float(W), None, op0=ALU.mult)
        nc.vector.tensor_add(n32f, n32f, offf)
        if stage < 4:
            continue

        # ---- phase 3 ----
        pagf = pw.tile([128, 8], F32, tag="pagf", name="pagf")
        nc.vector.tensor_scalar(pagf, n32f, sb[:, :], None, op0=ALU.add)
        idxC = wrap_idx(f"C", pagf, 8)
        pg = pw.tile([128, 8, 128], F32, tag="pg", name="pg")
        nc.gpsimd.dma_gather(pg[:, :, :], dr[f"pag_{h}"][:, :],
                             idxC[:, :], 1024, 1024, 128)

        qb = qt[h][:, :].rearrange("p (one d) -> p one d", one=1) \
                        .to_broadcast([128, 8, D])
        gb = gt[h][:, :].rearrange("p (one d) -> p one d", one=1) \
                        .to_broadcast([128, 8, D])
        diff = pw.tile([128, 8, D], F32, tag="diff", name="diff")
        nc.vector.tensor_sub(diff, qb, pg[:, :, 0:D])
        prod = pw.tile([128, 8, D], F32, tag="prod", name="prod")
        nc.vector.tensor_mul(prod, diff, gb)
        contrib = pw.tile([128, 8], F32, tag="contrib", name="contrib")
        nc.vector.reduce_sum(contrib, prod[:, :, :], axis=AX.X)
        nc.vector.tensor_mul(prod, diff, diff)
        d2 = pw.tile([128, 8], F32, tag="d2", name="d2")
        nc.vector.reduce_sum(d2, prod[:, :, :], axis=AX.X)
        dist = pw.tile([128, 8], F32, tag="dist", name="dist")
        nc.scalar.sqrt(dist, d2)
        adjm = pw.tile([128, 8, 64], F32, tag="adjm", name="adjm")
        nc.vector.tensor_mul(
            adjm, pg[:, :, D:128],
            eye[:, :].rearrange("p (one j) -> p one j", one=1)
                     .to_broadcast([128, 8, 64]))
        adjv = pw.tile([128, 8], F32, tag="adjv", name="adjv")
        nc.vector.reduce_sum(adjv, adjm[:, :, :], axis=AX.X)
        nc.vector.tensor_scalar(adjv, adjv, lw[:, :], None, op0=ALU.mult)
        nc.vector.tensor_add(adjv, adjv, contrib)
        # adjv now holds logits [128, 8]
        dfull = pw.tile([BLOC, 2 * K + NUM_SENT], F32, tag="dfull",
                        name="dfull")
        lfull = pw.tile([BLOC, 2 * K + NUM_SENT], F32, tag="lfull",
                        name="lfull")
        nc.vector.tensor_copy(dfull[:, 0:8], dist[0:BLOC, :])
        nc.sync.dma_start(dfull[:, 8:16], dist[BLOC:128, :])
        nc.vector.memset(dfull[:, 16:24], SENT_DIST)
        nc.vector.tensor_copy(lfull[:, 0:8], adjv[0:BLOC, :])
        nc.sync.dma_start(lfull[:, 8:16], adjv[BLOC:128, :])
        nc.vector.memset(lfull[:, 16:24], 0.0)

        mn = pw.tile([BLOC, 1], F32, tag="mn", name="mn")
        nc.vector.tensor_reduce(mn, dfull[:, :], axis=AX.X, op=ALU.min)
        e24 = pw.tile([BLOC, 24], F32, tag="e24", name="e24")
        nc.scalar.activation(e24, dfull[:, :], ACTF.Exp, bias=mn[:, :],
                             scale=-1.0)
        z = pw.tile([BLOC, 1], F32, tag="z", name="z")
        nc.vector.reduce_sum(z, e24[:, :], axis=AX.X)
        el = pw.tile([BLOC, 24], F32, tag="el", name="el")
        nc.vector.tensor_mul(el, e24, lfull)
        wl = pw.tile([BLOC, 1], F32, tag="wl", name="wl")
        nc.vector.reduce_sum(wl, el[:, :], axis=AX.X)
        rz = pw.tile([BLOC, 1], F32, tag="rz", name="rz")
        nc.vector.reciprocal(rz, z)
        nc.vector.tensor_mul(Racc[:, h:h + 1], wl, rz)

    if stage < 4:
        dbg = pf.tile([128, 1], F32, tag="dbg", name="dbg")
        nc.vector.tensor_copy(dbg, pooled[:, 0:1])
        nc.sync.dma_start(pred[:, :], dbg[0:BLOC, :])
        return pred, dr
    rs = pf.tile([BLOC, 1], F32, tag="rs", name="rs")
    nc.vector.reduce_sum(rs, Racc[:, :], axis=AX.X)
    outv = pf.tile([BLOC, 1], F32, tag="outv", name="outv")
    nc.scalar.activation(outv, rs, ACTF.Sigmoid, scale=1.0 / H)
    nc.sync.dma_start(pred[:, :], outv)
    return pred, dr


def build_nc(n_nodes=N_NODES, use_f32r=USE_F32R, pn2_bf16=PN2_BF16, stage=99):
    from contextlib import ExitStack
    nc = bacc.Bacc("TRN2", target_bir_lowering=False, debug=False)
    with tile.TileContext(nc) as tc:
        with ExitStack() as ctx:
            tc._emit_ctx = ctx
            emit(nc, tc, n_nodes, stage=stage)
    nc.compile()
    return nc


def host_prep(core, pos, grads, adj, label_w, edges, n_nodes=N_NODES,
              pn2_bf16=PN2_BF16):
    import ml_dtypes
    bf = ml_dtypes.bfloat16
    b0 = core * BLOC
    src = np.asarray(edges[0, b0:b0 + BLOC], np.int64)
    dst = np.asarray(edges[1, b0:b0 + BLOC], np.int64)
    pn2 = np.sum(pos * pos, axis=2)  # (N, H) f32

    im = {}
    for h in range(H):
        r = np.empty((65, n_nodes), np.float32)
        r[0:64] = pos[:, h, :].T
        r[64] = -pn2[:, h]
        im[f"rhs_{h}"] = r.astype(bf)
        L = np.empty((65, 128), np.float32)
        L[0:64, 0:64] = 2.0 * pos[src, h, :].T
        L[0:64, 64:128] = 2.0 * pos[dst, h, :].T
        L[64, :] = 1.0
        im[f"lhsT_{h}"] = L.astype(bf)
        q = np.empty((128, D), np.float32)
        q[0:64] = pos[src, h, :]
        q[64:128] = pos[dst, h, :]
        im[f"q_{h}"] = q
        g = np.empty((128, D), np.float32)
        g[0:64] = grads[dst, h, :]
        g[64:128] = grads[src, h, :]
        im[f"g_{h}"] = g
    adjcol = np.ascontiguousarray(adj[:, dst])          # (N, 64)
    adjrT = np.ascontiguousarray(adj[src, :].T)         # (N, 64)
    for h in range(H):
        pag = np.empty((2 * n_nodes, 128), np.float32)
        pag[0:n_nodes, 0:64] = pos[:, h, :]
        pag[0:n_nodes, 64:128] = adjcol
        pag[n_nodes:, 0:64] = pos[:, h, :]
        pag[n_nodes:, 64:128] = adjrT
        im[f"pag_{h}"] = pag
    im["l2"] = np.tile(np.eye(16, dtype=np.float32), (8, 8))
    gmv = np.zeros((128, 64), np.float32)
    kk = np.arange(128)
    for c in range(8):
        gmv[kk, c * 8 + kk // 16] = 1.0
    im["gm"] = gmv
    ey = np.zeros((128, 128), np.float32)
    ey[kk, kk % 64] = 1.0
    im["eye"] = ey
    sbv = np.zeros((128, 1), np.float32)
    sbv[64:, 0] = float(n_nodes)
    im["sb"] = sbv
    im["lw"] = np.full((128, 1), float(np.asarray(label_w).reshape(-1)[0]),
                       np.float32)
    return im


_NC_CACHE = {}


def kernel(pos, grads, adj, label_w, edges):
    pos = np.asarray(pos, np.float32)
    grads = np.asarray(grads, np.float32)
    adj = np.asarray(adj, np.float32)
    label_w = np.asarray(label_w, np.float32)
    edges_np = np.asarray(edges)

    key = (N_NODES,)
    if key not in _NC_CACHE:
        _NC_CACHE[key] = build_nc(N_NODES)
    nc = _NC_CACHE[key]

    in_maps = [host_prep(r, pos, grads, adj, label_w, edges_np, N_NODES)
               for r in range(N_CORES)]
    res = run_bass_kernel_spmd(nc, in_maps, core_ids=list(range(N_CORES)))
    out = np.concatenate([res.results[r]["pred"][:, 0]
                          for r in range(N_CORES)])
    return out.astype(np.float32)
